# revision 35
# baseline (speedup 1.0000x reference)
"""AdaFace loss on 8 TRN2 NeuronCores, class-parallel.

Baseline skeleton (one PSUM consumer per tile keeps the PE at full
clock): shard 100k weight rows across 8 cores, fp8e4 DoubleRow matmuls
with the whole weight shard resident in SBUF, fixed log-softmax shift
of 32 (|logit| <= 32) so no max pass / collective is needed.

v5 offload: per batch chunk, one of the six 2048-wide class tiles is
drained by the Vector engine as a Schraudolph fake-exp (one
tensor_scalar affine f32->int16; the int16 bit pattern IS bf16(exp)),
instead of by ScalarE. GpSimd absorbs one accumulate (eacc2 = fake +
one ScalarE tile) so the Vector engine's running-add chain stays flat.
ScalarE drops from 28 to 24 activations. Host folds the fake-exp mean
bias into the affine constant (rho de-bias) and does the O(B) combine:
margin-target correction, ln, weighted mean.
"""

import numpy as np
import ml_dtypes

import concourse.bass as bass
import concourse.tile as tile
from concourse import bacc, mybir
from concourse.bass_utils import run_bass_kernel_spmd

B = 512
D = 256
C = 100000
NCORES = 8
CSH = C // NCORES          # 12500 classes per core
CPAD = 12544
NPAD_TOT = (CPAD - CSH) * NCORES

M0 = 0.5
M_MIN = 0.25
SCALE = 32.0
SHIFT = 32.0               # fixed log-softmax shift (|logits| <= SCALE)
FP8_PRESCALE = 8.0         # both operands scaled by 8 -> matmul gives 64*cos

# tile order per chunk: small tile first (cheap pipeline starter), then
# six 2048-wide tiles; FAKE_BI selects which big tile the vector engine
# drains via fake-exp
TILES_SMALL_FIRST = [(12288, 256)] + [(i * 2048, 2048) for i in range(6)]
TILES_SMALL_LAST = [(i * 2048, 2048) for i in range(6)] + [(12288, 256)]
FAKE_C0 = 2 * 2048          # class offset of the fake-drained tile
GP_C0 = 3 * 2048            # ScalarE tile whose accumulate gpsimd takes

# Schraudolph constants: from x = 64*cos want bf16 bits of
# exp(0.5*x - 32): i16 = rint(x*FA + FB_EFF)
LOG2E = 1.4426950408889634
FA = 64.0 * LOG2E
FB = 16256.0 - 4096.0 * LOG2E

f32 = mybir.dt.float32
bf16 = mybir.dt.bfloat16
i16 = mybir.dt.int16
fp8 = mybir.dt.float8e4

NBC = B // 128             # 4 batch chunks

_cached_nc = None
_last_results = None


def _schraudolph_rho(fb):
    """Mean ratio fake_exp/exp over the logit range (HW rounds to
    nearest on the f32->i16 convert)."""
    t = np.linspace(-60.0, -1.0, 200001)
    x = (t + 32.0) * 2.0
    y = np.float32(x) * np.float32(FA) + np.float32(fb)
    i = np.rint(y).astype(np.int16)
    v = i.view(ml_dtypes.bfloat16).astype(np.float64)
    return float(np.mean(v / np.exp(t)))


# value ~ 2^((i-16256)/128): dividing by rho shifts the constant by
# -128*log2(rho)
FB_EFF = FB - 128.0 * np.log2(_schraudolph_rho(FB))
FB_EFF = FB_EFF - 128.0 * np.log2(_schraudolph_rho(FB_EFF))


def _build():
    global _cached_nc
    if _cached_nc is not None:
        return _cached_nc

    nc = bacc.Bacc(
        "TRN2", target_bir_lowering=False, debug=False, num_devices=NCORES
    )

    # [p, c, j] pair-interleaved fp8: contraction index k = j*128 + p
    wnT_d = nc.dram_tensor("wnT", [128, CPAD, 2], fp8, kind="ExternalInput")
    featnT_d = nc.dram_tensor("featnT", [128, 2, B], fp8, kind="ExternalInput")
    out_d = nc.dram_tensor("out", [128, NBC], f32, kind="ExternalOutput")

    with tile.TileContext(nc) as tc:
        with (
            tc.tile_pool(name="persist", bufs=1) as persist,
            tc.tile_pool(name="epool", bufs=3) as epool,
            tc.tile_pool(name="psum", bufs=2, space="PSUM") as psum,
        ):
            fsb = persist.tile([128, 2, B], fp8)
            nc.sync.dma_start(out=fsb[:], in_=featnT_d[:])

            wsb = persist.tile([128, CPAD, 2], fp8)
            # chunked loads in consumption order; sync's queue is free
            # earliest (shortest preamble), scalar joins after its
            # activation-table load
            plan = [
                (nc.sync, 12288, 12544),
                (nc.sync, 0, 1536),
                (nc.scalar, 1536, 3072),
                (nc.sync, 3072, 4608),
                (nc.scalar, 4608, 6144),
                (nc.sync, 6144, 7680),
                (nc.scalar, 7680, 9216),
                (nc.sync, 9216, 10752),
                (nc.scalar, 10752, 12288),
            ]
            for eng, lo, hi in plan:
                eng.dma_start(out=wsb[:, lo:hi, :], in_=wnT_d[:, lo:hi, :])

            bias_s = persist.tile([128, 1], f32)
            nc.gpsimd.memset(bias_s[:], -SHIFT)

            eacc = [
                persist.tile(
                    [128, 2048], bf16, tag=f"eacc{bc}", name=f"eacc{bc}"
                )
                for bc in range(NBC)
            ]

            S_all = persist.tile([128, NBC], f32)
            S_main = persist.tile([128, 1], f32)

            S_small = persist.tile([128, 1], f32)

            # per-tile roles by class offset:
            #   SMALL 12288 -> ScalarE, joins eacc[:, :256]
            #   b0 0        -> ScalarE, starts eacc
            #   b1 2048     -> ScalarE, gpsimd pairs it with b4 (ea3)
            #   b2 4096     -> FAKE: vector-engine Schraudolph drain (fi)
            #   b3 6144     -> ScalarE, gpsimd pairs it with fi (ea2)
            #   b4 8192     -> ScalarE, into ea3
            #   b5 10240    -> ScalarE, vector adds it + ea2 + ea3
            for bc in range(NBC):
                last = bc == NBC - 1
                tiles = TILES_SMALL_LAST if last else TILES_SMALL_FIRST
                lhs = fsb[:, :, bc * 128:(bc + 1) * 128]
                fi = epool.tile([128, 2048], i16, tag="fi")
                ea2 = epool.tile([128, 2048], bf16, tag="ea2")
                ea3 = epool.tile([128, 2048], bf16, tag="ea3")
                esc_by_c0 = {}
                for ti, (c0, cw) in enumerate(tiles):
                    ps = psum.tile([128, 2048], f32, tag="ps")
                    for j in range(0, cw, 512):
                        jw = min(512, cw - j)
                        nc.tensor.matmul(
                            ps[:, j:j + jw],
                            lhs,
                            wsb[:, c0 + j:c0 + j + jw, :].transpose([0, 2, 1]),
                            start=True, stop=True,
                            perf_mode=mybir.MatmulPerfMode.DoubleRow,
                        )
                    if c0 == FAKE_C0:
                        nc.vector.tensor_scalar(
                            fi[:], ps[:],
                            FA, FB_EFF,
                            mybir.AluOpType.mult, mybir.AluOpType.add,
                        )
                        continue
                    esc = epool.tile([128, cw], bf16, tag=f"esc{c0}")
                    esc_by_c0[c0] = esc
                    nc.scalar.activation(
                        esc[:], ps[:, :cw],
                        mybir.ActivationFunctionType.Exp,
                        bias=bias_s[:], scale=SCALE / (FP8_PRESCALE**2),
                    )
                    if c0 == 12288 and last:   # small tile at chunk end
                        nc.vector.tensor_add(
                            eacc[bc][:, :cw], eacc[bc][:, :cw], esc[:]
                        )
                    elif c0 == 2048:         # b1: gpsimd folds b0 + b1
                        nc.gpsimd.tensor_tensor(
                            ea3[:], esc_by_c0[0][:], esc[:],
                            mybir.AluOpType.add,
                        )
                    elif c0 == GP_C0:        # b3: gpsimd folds fi + b3
                        nc.gpsimd.tensor_tensor(
                            ea2[:], fi[:].bitcast(bf16), esc[:],
                            mybir.AluOpType.add,
                        )
                    elif c0 == 8192:         # b4: starts the accumulator
                        nc.vector.tensor_copy(eacc[bc][:], esc[:])
                        if not last:
                            nc.vector.tensor_add(
                                eacc[bc][:, :256], eacc[bc][:, :256],
                                esc_by_c0[12288][:],
                            )
                    elif c0 == 10240:        # b5
                        nc.vector.tensor_add(eacc[bc][:], eacc[bc][:], esc[:])
                        nc.vector.tensor_add(eacc[bc][:], eacc[bc][:], ea3[:])
                        nc.vector.tensor_add(eacc[bc][:], eacc[bc][:], ea2[:])
                        if last:
                            # early partial reduce: everything but the
                            # small tile's [0:256] region
                            nc.vector.tensor_reduce(
                                S_main[:],
                                eacc[bc][:, 256:],
                                axis=mybir.AxisListType.X,
                                op=mybir.AluOpType.add,
                            )
                    if bc > 0 and ti == 1:
                        nc.vector.tensor_reduce(
                            S_all[:, bc - 1:bc],
                            eacc[bc - 1][:],
                            axis=mybir.AxisListType.X,
                            op=mybir.AluOpType.add,
                        )

            nc.vector.tensor_reduce(
                S_small[:],
                eacc[NBC - 1][:, :256],
                axis=mybir.AxisListType.X,
                op=mybir.AluOpType.add,
            )
            nc.vector.tensor_add(S_all[:, NBC - 1:NBC], S_main[:], S_small[:])
            nc.sync.dma_start(out=out_d[:], in_=S_all[:])

    nc.compile()
    _cached_nc = nc
    return nc


def _host_prep(features, weight, weights, labels):
    """Everything O(B*D) / O(C*D) that is not the big matmul."""
    f = features.astype(np.float64)
    norms = np.sqrt((f * f).sum(axis=1))
    lo, hi = norms.min(), norms.max()
    denom = max(hi - lo, 1e-8)
    margins = np.clip(M_MIN + (M0 - M_MIN) * (norms - lo) / denom, M_MIN, M0)
    feat_n = f / np.maximum(norms, 1e-12)[:, None]

    wlab = weight[labels].astype(np.float64)
    wlab_n = wlab / np.maximum(
        np.sqrt((wlab * wlab).sum(axis=1)), 1e-12
    )[:, None]
    cos_t = np.clip((feat_n * wlab_n).sum(axis=1), -1.0 + 1e-7, 1.0 - 1e-7)
    cos_m = cos_t * np.cos(margins) - np.sqrt(1.0 - cos_t * cos_t) * np.sin(
        margins
    )
    t_logit = SCALE * cos_m
    corr = (
        np.exp(SCALE * cos_m - SHIFT)
        - np.exp(SCALE * cos_t - SHIFT)
        - NPAD_TOT * np.exp(-SHIFT)
    )
    coef = weights.astype(np.float64) / B
    return feat_n, corr, coef, t_logit


def _to_dr_layout(mat_t, width):
    """[D, X] f32 -> [128, X, 2] fp8, pair-interleaved, k = j*128 + p."""
    a = mat_t.reshape(2, 128, width)          # [j, p, X]
    a = np.ascontiguousarray(a.transpose(1, 2, 0))  # [p, X, j]
    return a.astype(ml_dtypes.float8_e4m3)


def kernel(features, weight, weights, labels):
    global _last_results
    features = np.asarray(features, dtype=np.float32)
    weight = np.asarray(weight, dtype=np.float32)
    weights = np.asarray(weights, dtype=np.float32)
    labels = np.asarray(labels).astype(np.int64)

    feat_n, corr, coef, t_logit = _host_prep(features, weight, weights, labels)

    wn = weight / np.maximum(
        np.linalg.norm(weight, axis=1, keepdims=True), 1e-12
    )
    featnT = np.ascontiguousarray(feat_n.T.astype(np.float32)) * FP8_PRESCALE
    a = featnT.reshape(2, 128, B)
    featnT8 = np.ascontiguousarray(a.transpose(1, 0, 2)).astype(
        ml_dtypes.float8_e4m3
    )

    in_maps = []
    for i in range(NCORES):
        sh = wn[i * CSH:(i + 1) * CSH]  # [CSH, D]
        wt = np.zeros((D, CPAD), dtype=np.float32)
        wt[:, :CSH] = sh.T * FP8_PRESCALE
        in_maps.append(
            {"wnT": _to_dr_layout(wt, CPAD), "featnT": featnT8}
        )

    nc = _build()
    res = run_bass_kernel_spmd(nc, in_maps, list(range(NCORES)))
    _last_results = res

    # ---- host combine ----
    S = np.zeros(B, dtype=np.float64)
    for i in range(NCORES):
        sc = np.asarray(res.results[i]["out"], dtype=np.float64)  # [128, 4]
        for bc in range(NBC):
            S[bc * 128:(bc + 1) * 128] += sc[:, bc]

    Z = S + corr
    per = SHIFT + np.log(Z) - t_logit
    loss = float((coef * per).sum())
    return np.array(loss, dtype=np.float32)
